# revision 44
# baseline (speedup 1.0000x reference)
"""BandSplit kernel for 8x Trainium2 NeuronCores — host-rstd design.

Math (per batch b, band n, time t):
    feat vector v[64] gathered from (x_real, x_imag) at freqs n*16..n*16+15
    ln  = (v - mean(v)) * rsqrt(var(v)+eps) * ln_w[n] + ln_b[n]
    z[b,n,t,:] = ln @ fc_w[n] + fc_b[n]

Key identity (mean-fold):  z = rstd(n,t) * (x @ W'') + b'  exactly, where
    W''[n,f,e] = ln_w[n,f]*fc_w[n,f,e] - (1/64)*sum_g ln_w[n,g]*fc_w[n,g,e]
    b'[n,e]    = fc_b[n,e] + sum_f ln_b[n,f]*fc_w[n,f,e]

Device decomposition (data-parallel over batch, one b per core):
    - The device computes ONLY y = x @ W'' (bf16) and the per-(band,t) raw
      sums mean = E[x], m2 = E[x^2]; rstd and the affine (rstd*y + b') are
      applied on the host during the unpack it already performs. This kills
      the rstd-broadcast matmuls, the xh vector multiplies and the scalar
      Ln/Exp chain of the previous design.
    - Stats: per tile j a [128,64] selector with 1/64 at cols (2j, 2j+1)
      accumulates (start=j==0, stop=j==31) band sums into 4 persistent psum
      accumulators (mean/m2 x t-half), rows 2j/2j+1. One drain + one 512KB
      DMA at the end ships all stats.
    - Main GEMM W-stationary, band-outer/t-half-inner so each weight load
      serves 2 matmuls. Even band = partitions 0-63, odd = 64-127 (separate
      PE-array tiles via tile_position).
    - z psum drains alternate scalar (ACTIVATE Copy) / vector (tensor_copy)
      so neither engine gates the PE's psum ring.
    - x packed bf16 on host (halves input HBM traffic); z stored bf16
      [128 e, 2048 (band,t)] per tile; W deduplicated [128, 32*128] (even
      band blocks on partitions 0-63, odd on 64-127) = 1 MB.
    - ALL DMAs issued from the (otherwise idle) sync engine's HW-DGE queue.
    - PSUM: 4 stats accumulators + 4-deep z ring = 8 banks exactly.
    - This toolchain allows ONE semaphore wait per instruction; extras are
      hoisted onto NoOps by _legalize_waits.
"""

import numpy as np
import ml_dtypes
from contextlib import ExitStack

import concourse.bass as bass
import concourse.tile as tile
from concourse import mybir
from concourse.bass_utils import run_bass_kernel_spmd

B, C, F, T = 8, 2, 1024, 1024
NB, BW, EMB = 64, 16, 128
FEAT = C * BW * 2  # 64
EPS = 1e-5
NCORES = 8
NTILES = NB // 2   # 32 band-pair tiles per core
GROUP = 4
Nb_GROUPS = NTILES // GROUP

f32 = mybir.dt.float32
bf16 = mybir.dt.bfloat16


def _build_kernel(ctx, tc, xarr, wcomb, onesall, zdev, sdev):
    nc = tc.nc
    AF = mybir.ActivationFunctionType
    const = ctx.enter_context(tc.tile_pool(name="const", bufs=1))
    xpool = ctx.enter_context(tc.tile_pool(name="xpool", bufs=4 * GROUP))
    sqpool = ctx.enter_context(tc.tile_pool(name="sqpool", bufs=3 * GROUP))
    zpool = ctx.enter_context(tc.tile_pool(name="zpool", bufs=4))
    stg = ctx.enter_context(tc.tile_pool(name="stg", bufs=1))
    pstat = ctx.enter_context(tc.tile_pool(name="pstat", bufs=1, space="PSUM"))
    pz = ctx.enter_context(tc.tile_pool(name="pz", bufs=6, space="PSUM"))

    # W'' blocks: even band j -> partitions 0-63 at cols j*128, odd -> 64-127
    W_sb = const.tile([128, NTILES * EMB], bf16)
    # per-tile stats selectors: [128, 64] slice j has 1/64 at (rows 0-63,
    # col 2j) and (rows 64-127, col 2j+1); aligned slices (a sliding-window
    # variant measured ~4.5us slower, likely misaligned weight loads)
    ones_sb = const.tile([128, NTILES * 64], bf16)

    # 2 persistent stats accumulator banks (one per t-half): partitions
    # 0-63 hold the mean sums (PE-array cols 0-63), partitions 64-127 the
    # E[x^2] sums (cols 64-127 via tile_position=(0,64))
    P = [pstat.tile([128, 512], f32, name=f"pstat{k}") for k in range(2)]

    def load_consts_early():
        nc.sync.dma_start(ones_sb[:], onesall[:])

    def load_w_quarter(q):
        c0 = q * (NTILES * EMB // 4)
        c1 = (q + 1) * (NTILES * EMB // 4)
        nc.sync.dma_start(W_sb[:, c0:c1], wcomb[:, c0:c1])

    def x_load(g, only=None):
        xs = []
        for i in range(GROUP):
            if only is not None and i not in only:
                xs.append(None)
                continue
            xt = xpool.tile([128, T], bf16, tag="xt")
            nc.sync.dma_start(xt[:], xarr[g * GROUP + i])
            xs.append(xt)
        return xs

    def sq_comp(g, xs):
        # squares for E[x^2]: vector at startup, then 3 gpsimd + 1 scalar
        # per group keeps all three engines ~balanced
        sqs = []
        for i in range(GROUP):
            sq = sqpool.tile([128, T], bf16, tag="sq")
            if g == 0:
                nc.vector.tensor_mul(sq[:], xs[i][:], xs[i][:])
            elif i == 3:
                nc.scalar.activation(sq[:], xs[i][:], mybir.ActivationFunctionType.Square)
            else:
                nc.gpsimd.tensor_mul(sq[:], xs[i][:], xs[i][:])
            sqs.append(sq)
        return sqs

    def stats_m(j, xt):
        # mean sums -> P[h] rows 0-63 (accumulating rows 2j/2j+1)
        osl = ones_sb[:, j * 64 : (j + 1) * 64]
        s0, st = (j == 0), (j == NTILES - 1)
        for h in range(2):
            nc.tensor.matmul(
                P[h][0:64, :], osl, xt[:, h * 512 : h * 512 + 512],
                start=s0, stop=st, skip_group_check=True, tile_position=(0, 0),
            )

    def stats_e(j, sqt):
        # E[x^2] sums -> P[h] rows 64-127; psum accumulation is commutative
        # so these may be emitted later than the m-mms of the same tile
        osl = ones_sb[:, j * 64 : (j + 1) * 64]
        s0, st = (j == 0), (j == NTILES - 1)
        for h in range(2):
            nc.tensor.matmul(
                P[h][64:128, :], osl, sqt[:, h * 512 : h * 512 + 512],
                start=s0, stop=st, skip_group_check=True, tile_position=(0, 64),
            )

    def tile_work(j, xt, sqt, defer_e=False):
        stats_m(j, xt)
        if not defer_e:
            stats_e(j, sqt)
        # main GEMM: band-outer so each W load serves both t-halves; 6-deep
        # single-bank psum ring, drains alternating scalar/vector
        zst = zpool.tile([128, 2 * T], bf16, tag="zst")
        for bl in range(2):
            p0 = 64 * bl
            wsl = W_sb[p0 : p0 + 64, j * EMB : (j + 1) * EMB]
            for h in range(2):
                pzt = pz.tile([128, 512], f32, tag="pz")
                nc.tensor.matmul(
                    pzt[:], wsl, xt[p0 : p0 + 64, h * 512 : h * 512 + 512],
                    tile_position=(p0, 0),
                )
                c0 = bl * T + h * 512
                if h == 0:
                    nc.scalar.activation(zst[:, c0 : c0 + 512], pzt[:], AF.Copy)
                else:
                    nc.vector.tensor_copy(zst[:, c0 : c0 + 512], pzt[:])
        nc.sync.dma_start(zdev[j], zst[:])

    def stats_drain():
        # stage rows: 0-63 mean (band 2j+r at row 2j+r), 64-127 E[x^2];
        # cols h*512+t'
        stage = stg.tile([128, 2 * 512], f32)
        for h in range(2):
            nc.scalar.activation(stage[:, h * 512 : (h + 1) * 512], P[h][:], AF.Copy)
        nc.sync.dma_start(sdev[:], stage[:])

    # startup: first x tile + selectors + W quarter 0 lead the queue so the
    # first stats AND main matmuls can go as early as possible; group 0's
    # e-stats are deferred past its mains (accumulation commutes) so the
    # first PE work never waits on sq0
    xs = {0: x_load(0, only=(0,))}
    load_consts_early()
    load_w_quarter(0)
    x1 = x_load(0, only=(1, 2, 3))
    xs[0] = [xs[0][0], x1[1], x1[2], x1[3]]
    if Nb_GROUPS > 1:
        xs[1] = x_load(1)
    for q in range(1, 4):
        load_w_quarter(q)
    sqs = {0: sq_comp(0, xs[0])}

    for g in range(Nb_GROUPS):
        if g + 2 < Nb_GROUPS:
            xs[g + 2] = x_load(g + 2)
        if g + 1 < Nb_GROUPS:
            sqs[g + 1] = sq_comp(g + 1, xs[g + 1])
        for i in range(GROUP):
            tile_work(g * GROUP + i, xs[g][i], sqs[g][i], defer_e=(g == 0))
        if g == 0:
            for i in range(GROUP):
                stats_e(i, sqs[0][i])
        del xs[g], sqs[g]
    stats_drain()


def _legalize_waits(nc):
    """walrus here accepts ONE sync-wait per instruction; hoist extras onto
    single-wait NoOps inserted just before (same engine, same semantics)."""
    n_split = 0
    for f in nc.m.functions:
        for blk in f.blocks:
            newlist = []
            for ins in blk.instructions:
                si = ins.sync_info
                if si is not None and len(si.on_wait) > 1:
                    waits = list(si.on_wait)
                    for w in waits[:-1]:
                        nop = mybir.InstEventSemaphore(
                            name=f"{ins.name}-w{n_split}",
                            ins=[],
                            outs=[],
                            engine=ins.engine,
                        )
                        nop.sync_info = mybir.SyncInfo(on_wait=[w], on_update=[])
                        newlist.append(nop)
                        n_split += 1
                    ins.sync_info = mybir.SyncInfo(
                        on_wait=[waits[-1]], on_update=list(si.on_update)
                    )
                newlist.append(ins)
            blk.instructions = newlist
    return n_split


def build_nc(legalize=True):
    nc = bass.Bass("TRN2", target_bir_lowering=False, debug=False)
    xarr = nc.dram_tensor("xarr", [NTILES, 128, T], bf16, kind="ExternalInput")
    wcomb = nc.dram_tensor("wcomb", [128, NTILES * EMB], bf16, kind="ExternalInput")
    onesall = nc.dram_tensor("onesall", [128, NTILES * 64], bf16, kind="ExternalInput")
    zdev = nc.dram_tensor("zdev", [NTILES, 128, 2 * T], bf16, kind="ExternalOutput")
    sdev = nc.dram_tensor("sdev", [128, 2 * 512], f32, kind="ExternalOutput")
    with tile.TileContext(nc) as tc, ExitStack() as ctx:
        _build_kernel(ctx, tc, xarr.ap(), wcomb.ap(), onesall.ap(), zdev.ap(), sdev.ap())
    if legalize:
        _legalize_waits(nc)
    return nc


_NC = None


def _get_nc():
    global _NC
    if _NC is None:
        _NC = build_nc()
    return _NC


def _install_ntff_shim():
    """Register the axon NTFF profile hook (dev/testing only; the image's
    antenv package lacks axon_hooks, so bass_utils trace=True would fail)."""
    import sys
    import types

    if "antenv.axon_hooks" in sys.modules:
        return
    mod = types.ModuleType("antenv.axon_hooks")
    mod._hook = None

    def set_axon_ntff_profile_hook(h):
        mod._hook = h

    def get_axon_ntff_profile_hook():
        return mod._hook

    mod.set_axon_ntff_profile_hook = set_axon_ntff_profile_hook
    mod.get_axon_ntff_profile_hook = get_axon_ntff_profile_hook
    sys.modules["antenv.axon_hooks"] = mod
    try:
        import antenv

        antenv.axon_hooks = mod
    except ImportError:
        pass
    try:
        from trn_agent_boot.trn_boot import _ntff_profile_via_ctypes

        mod._hook = _ntff_profile_via_ctypes("/opt/axon/libaxon_pjrt.so")
    except Exception as e:
        print(f"ntff shim: no hook ({e})")


def _host_params(ln_w, ln_b, fc_w, fc_b):
    lw = ln_w.astype(np.float64)
    lb = ln_b.astype(np.float64)
    fw = fc_w.astype(np.float64)
    fb = fc_b.astype(np.float64)
    W1 = lw[:, :, None] * fw                          # [NB, FEAT, EMB]
    Wpp = W1 - W1.sum(1, keepdims=True) / FEAT        # mean-fold
    bp = fb + (lb[:, :, None] * fw).sum(1)            # [NB, EMB]
    # partition q = i*32 + c*16 + w  maps to feat index c*32 + w*2 + i
    q = np.arange(FEAT)
    i = q // 32
    c = (q % 32) // 16
    w = q % 16
    perm = c * 32 + w * 2 + i
    Wq = Wpp[:, perm, :]                              # [NB, 64(q), EMB]
    wcomb_np = np.zeros((128, NTILES * EMB), np.float64)
    for j in range(NTILES):
        wcomb_np[0:64, j * EMB : (j + 1) * EMB] = Wq[2 * j]
        wcomb_np[64:128, j * EMB : (j + 1) * EMB] = Wq[2 * j + 1]
    wcomb_np = wcomb_np.astype(ml_dtypes.bfloat16)
    bp_np = bp.astype(np.float32)                     # [NB, EMB]
    ones_np = np.zeros((128, NTILES * 64), np.float32)
    for j in range(NTILES):
        ones_np[0:64, j * 64 + 2 * j] = 1.0 / FEAT
        ones_np[64:128, j * 64 + 2 * j + 1] = 1.0 / FEAT
    ones_np = ones_np.astype(ml_dtypes.bfloat16)
    return wcomb_np, bp_np, ones_np


def _pack_x(x_real, x_imag):
    """[B,C,F,T] x2 -> [B, NTILES, 128, T] bf16: partition q = i*32+c*16+w."""
    xr = x_real.reshape(B, C, NTILES, 2, BW, T)   # [b, c, tile, bp, w, t]
    xi = x_imag.reshape(B, C, NTILES, 2, BW, T)
    out = np.empty((B, NTILES, 2, 2, C, BW, T), np.float32)
    out[:, :, :, 0] = xr.transpose(0, 2, 3, 1, 4, 5)
    out[:, :, :, 1] = xi.transpose(0, 2, 3, 1, 4, 5)
    return np.ascontiguousarray(
        out.reshape(B, NTILES, 128, T).astype(ml_dtypes.bfloat16)
    )


def _unpack_z(zdev, sdev, bp):
    """[B, NTILES, 128, 2048] bf16 + stats -> [B, NB, T, EMB] f32:
    z = rstd * y + b'. Stats: row 2j+r = band 2j+r mean (rows 0-63) or
    E[x^2] (rows 64-127), cols h*512+t'."""
    nb = zdev.shape[0]
    y = zdev.astype(np.float32).reshape(nb, NTILES, EMB, 2, T)
    y = y.transpose(0, 1, 3, 4, 2).reshape(nb, NB, T, EMB)  # [b, band, t, e]
    s = sdev.astype(np.float32)                              # [b, 128, 1024]
    mean = s[:, 0:64, :]
    m2 = s[:, 64:128, :]
    var = m2 - mean * mean
    rstd = 1.0 / np.sqrt(var + EPS)                          # [b, NB, T]
    z = y * rstd[:, :, :, None] + bp[None, :, None, :]
    return z


def kernel(x_real, x_imag, ln_w, ln_b, fc_w, fc_b, _trace=False):
    x_real = np.asarray(x_real, dtype=np.float32)
    x_imag = np.asarray(x_imag, dtype=np.float32)
    ln_w = np.asarray(ln_w, dtype=np.float32)
    ln_b = np.asarray(ln_b, dtype=np.float32)
    fc_w = np.asarray(fc_w, dtype=np.float32)
    fc_b = np.asarray(fc_b, dtype=np.float32)

    if _trace:
        _install_ntff_shim()
    wcomb_np, bp_np, ones_np = _host_params(ln_w, ln_b, fc_w, fc_b)
    xarr = _pack_x(x_real, x_imag)
    nc = _get_nc()
    in_maps = [
        {
            "xarr": xarr[i],
            "wcomb": wcomb_np,
            "onesall": ones_np,
        }
        for i in range(NCORES)
    ]
    res = run_bass_kernel_spmd(nc, in_maps, list(range(NCORES)), trace=_trace)
    if _trace and res.exec_time_ns is not None:
        print(f"HW exec time: {res.exec_time_ns} ns")
        if res.instructions_and_trace is not None:
            print(f"trace: {res.instructions_and_trace[1]}")
    zdev = np.stack([res.results[i]["zdev"] for i in range(NCORES)], axis=0)
    sdev = np.stack([res.results[i]["sdev"] for i in range(NCORES)], axis=0)
    return _unpack_z(zdev, sdev, bp_np)
